# revision 31
# baseline (speedup 1.0000x reference)
"""Trainium2 Bass kernel for nn_AttentionModule.

Computation (per batch row b, input feature i):
    E      = tanh(x @ E_W + E_b)                      # [B, 50]
    s      = einsum('be,iea->bia', E, A_W) + A_b      # [B, 66, 20]
    A      = softmax(s, -1)[..., 1]                   # [B, 66]
    out    = x * A

Key rewrite: softmax(s)[1] = 1 / sum_a exp(s_a - s_1).  Weights are
pre-differenced on the host (dW = A_W - A_W[:,:,1:2]), bias folded into the
matmul via a constant-1 row of E (tanh(30) == 1), giving
    den[b,i] = sum_a exp(s[b,i,a]),  out = x / den.

The den pipeline is elementwise-bound (~1320 exp+reduce elems per row), so
work is split across ALL engines two ways by feature index i:

S-part (i 0..15, row-oriented): mm2S emits s [128 rows, 320] per block;
  ACT does exact exp; DVE tensor_reduce over contiguous 20-col groups;
  DVE reciprocal; Pool multiplies x*rec.

T-part (i 16..65, column-oriented): mm2T emits s^T in 8 chunks of 125
  (i,a)-pairs x 256 rows; the drain to SBUF is split between ACT (exact
  exp) and DVE (Schraudolph exp: mm2 emits s' = A*s + B via host-scaled
  weights, so int32(s') bit-pattern IS ~exp(s), ~1.6% rms err); the
  GROUPED REDUCE RUNS ON THE PE as a matmul with a 0/1 membership matrix
  G, accumulating den^T [50, 512] in PSUM; DVE divides x^T/den^T (x^T is
  already on hand from the mm1 transpose); PE transposes out^T back and
  ACT copies it into the output tile.

Schraudolph safety: needs s >= -88 so s' stays >= 0 (else the int32 bit
pattern is garbage).  Real data has s in [-57, 55].  The f32r truncation
of the ~1e9 B in the bias row shifts all outputs by a small common factor
(~0.5-1%), well inside the 2e-2 gate.
"""

import numpy as np

B_TOTAL, INPUT, E_NODE, A_NODE = 262144, 66, 50, 20
N_CORES = 8
B_LOCAL = B_TOTAL // N_CORES          # 32768
NBLK = 4                              # 128-row blocks per macro tile
MACRO = 128 * NBLK                    # 512
NMACRO = B_LOCAL // MACRO             # 64
NIA = INPUT * A_NODE                  # 1320
CONST_ROW_BIAS = 30.0                 # tanh(30) == 1.0 in fp32

NI_S = 16                             # S-part features (row-oriented)
NI_T = INPUT - NI_S                   # 50 T-part features (col-oriented)
SCOLS = NI_S * A_NODE                 # 320
TCOLS = NI_T * A_NODE                 # 1000
NCH, CW = 8, 125                      # T chunks: 8 x 125 (i,a)-pairs
HALF = 256                            # T-part row-half size (PSUM fit)

# bf16-flavored Schraudolph: exp bits live in the TOP 16 bits of fp32, so
# mm2 emits s' = A*s + B at bf16 scale and int16(s') IS ~exp(s) as bf16
SCHRAU_A = float(2**7 / np.log(2.0))   # 184.664965
SCHRAU_B = 1064986822.0 / 65536        # 16250.41

TAIL_GROUP = 4           # blocks per S-part reduce/recip/mul op group
DMA_MACROS = 2           # macros per x-load/y-store DMA
TDRAIN_ACT = 288         # cols of each 512-col T-chunk-pair drained by ACT
                         # (exact exp); DVE Schraudolph-converts the rest
MM_F32R = True           # f32r mm1

_CACHE = {}


def _build_bass(n_rows, repeat=1):
    import concourse.bass as bass
    import concourse.bacc as bacc
    import concourse.tile as tile
    from concourse import mybir
    from concourse.masks import make_identity
    from contextlib import ExitStack

    f32 = mybir.dt.float32
    f32r = mybir.dt.float32r
    i16 = mybir.dt.int16
    bf16 = mybir.dt.bfloat16
    nmacro = n_rows // MACRO

    nc = bacc.Bacc("TRN2", target_bir_lowering=False, debug=False,
                   num_devices=N_CORES)

    x_d = nc.dram_tensor("x", [n_rows, INPUT], f32, kind="ExternalInput").ap()
    w1_d = nc.dram_tensor("W1", [INPUT, E_NODE + 1], f32, kind="ExternalInput").ap()
    b1_d = nc.dram_tensor("b1", [E_NODE + 1, 1], f32, kind="ExternalInput").ap()
    w2_d = nc.dram_tensor("W2", [E_NODE + 1, NIA], f32r, kind="ExternalInput").ap()
    g_d = nc.dram_tensor("G", [CW, NCH * NI_T], bf16, kind="ExternalInput").ap()
    y_d = nc.dram_tensor("y", [n_rows, INPUT], f32, kind="ExternalOutput").ap()

    x_r = x_d.rearrange("(m p) f -> m p f", p=128)
    y_r = y_d.rearrange("(m p) f -> m p f", p=128)

    with tile.TileContext(nc) as tc, ExitStack() as ctx:
        const = ctx.enter_context(tc.tile_pool(name="const", bufs=1))
        xpool = ctx.enter_context(tc.tile_pool(name="xp", bufs=3))
        xtp = ctx.enter_context(tc.tile_pool(name="xtp", bufs=2))
        etp = ctx.enter_context(tc.tile_pool(name="etp", bufs=2))
        expsp = ctx.enter_context(tc.tile_pool(name="expsp", bufs=3))
        exptp = ctx.enter_context(tc.tile_pool(name="exptp", bufs=6))
        denp = ctx.enter_context(tc.tile_pool(name="denp", bufs=6))
        outtp = ctx.enter_context(tc.tile_pool(name="outtp", bufs=3))
        outp = ctx.enter_context(tc.tile_pool(name="outp", bufs=3))
        ps_xt = ctx.enter_context(tc.tile_pool(name="ps_xt", bufs=1, space="PSUM"))
        ps_et = ctx.enter_context(tc.tile_pool(name="ps_et", bufs=1, space="PSUM"))
        ps_sS = ctx.enter_context(tc.tile_pool(name="ps_sS", bufs=2, space="PSUM"))
        ps_T = ctx.enter_context(tc.tile_pool(name="ps_T", bufs=2, space="PSUM"))
        ps_den = ctx.enter_context(tc.tile_pool(name="ps_den", bufs=1, space="PSUM"))
        ps_txb = ctx.enter_context(tc.tile_pool(name="ps_txb", bufs=1, space="PSUM"))

        w1_sb = const.tile([INPUT, E_NODE + 1], f32r if MM_F32R else f32)
        nc.sync.dma_start(out=w1_sb, in_=w1_d.bitcast(w1_sb.dtype))
        b1_sb = const.tile([E_NODE + 1, 1], f32)
        nc.sync.dma_start(out=b1_sb, in_=b1_d)
        w2_sb = const.tile([E_NODE + 1, NIA], f32r)
        nc.sync.dma_start(out=w2_sb, in_=w2_d)
        g_sb = const.tile([CW, NCH * NI_T], bf16)
        nc.sync.dma_start(out=g_sb, in_=g_d)
        g3 = g_sb.rearrange("p (c i) -> p c i", i=NI_T)
        w2T = w2_sb[:, SCOLS:NIA].rearrange("p (c w) -> p c w", w=CW)
        ident = const.tile([128, 128], f32)
        make_identity(nc, ident)
        exp_bias = const.tile([128, 1], f32)
        nc.vector.memset(exp_bias, -SCHRAU_B / SCHRAU_A)

        DM = DMA_MACROS
        assert nmacro % DM == 0
        iters = [m for _ in range(repeat) for m in range(nmacro)]
        xgs = {}

        def emit_load(git):
            m0 = iters[git * DM]
            xg = xpool.tile([128, DM * NBLK, INPUT], f32)
            nc.sync.dma_start(
                out=xg,
                in_=x_r[m0 * NBLK:m0 * NBLK + DM * NBLK]
                .rearrange("m p f -> p m f"),
            )
            return xg

        def emit_head(it):
            """PE transpose -> copy -> mm1 -> tanh for iteration it."""
            git, off = it // DM, it % DM
            if git not in xgs:
                xgs[git] = emit_load(git)
            x_sb = xgs[git][:, off * NBLK:(off + 1) * NBLK, :]
            xt_ps = ps_xt.tile([INPUT, MACRO], f32)
            for b in range(NBLK):
                nc.tensor.transpose(
                    xt_ps[:, b * 128:(b + 1) * 128], x_sb[:, b, :], ident
                )
            xt_sb = xtp.tile([INPUT, MACRO], f32r if MM_F32R else f32)
            nc.scalar.copy(out=xt_sb, in_=xt_ps)
            et_ps = ps_et.tile([E_NODE + 1, MACRO], f32)
            nc.tensor.matmul(et_ps, w1_sb, xt_sb, start=True, stop=True)
            et_sb = etp.tile([E_NODE + 1, MACRO], f32r)
            nc.scalar.activation(
                et_sb, et_ps, mybir.ActivationFunctionType.Tanh,
                bias=b1_sb, scale=1.0,
            )
            return x_sb, xt_sb, et_sb

        ogs = {}
        heads = {0: emit_head(0)}
        for it in range(len(iters)):
            git, off = it // DM, it % DM
            if it + 1 < len(iters):
                heads[it + 1] = emit_head(it + 1)
            x_sb, xt_sb, et_sb = heads.pop(it)

            if git not in ogs:
                ogs[git] = outp.tile([128, DM * NBLK, INPUT], f32, name="og")
            out_sb = ogs[git][:, off * NBLK:(off + 1) * NBLK, :]
            TG = TAIL_GROUP

            # ---- S-part (features NI_T..65): rows-oriented, DVE-reduced ----
            exp_gS = None
            for b in range(NBLK):
                bg = b % TG
                s_psS = ps_sS.tile([128, 512], f32)
                nc.tensor.matmul(
                    s_psS[:, 0:SCOLS], et_sb[:, b * 128:(b + 1) * 128],
                    w2_sb[:, 0:SCOLS], start=True, stop=True,
                )
                if bg == 0:
                    exp_gS = expsp.tile([128, TG * SCOLS], f32, name="exp_gS")
                nc.scalar.activation(
                    exp_gS[:, bg * SCOLS:(bg + 1) * SCOLS], s_psS[:, 0:SCOLS],
                    mybir.ActivationFunctionType.Exp,
                    scale=1.0 / SCHRAU_A, bias=exp_bias,
                )
                if bg != TG - 1:
                    continue
                b0 = b - bg
                g = exp_gS.rearrange("p (g a) -> p g a", a=A_NODE)
                den = denp.tile([128, TG * NI_S], f32, name="denS")
                rec = denp.tile([128, TG * NI_S], f32, name="recS")
                nc.vector.tensor_reduce(
                    out=den, in_=g,
                    axis=mybir.AxisListType.X, op=mybir.AluOpType.add,
                )
                nc.vector.reciprocal(out=rec, in_=den)
                nc.gpsimd.tensor_tensor(
                    out=out_sb[:, b0:b0 + TG, NI_T:INPUT],
                    in0=x_sb[:, b0:b0 + TG, NI_T:INPUT],
                    in1=rec.rearrange("p (t f) -> p t f", f=NI_S),
                    op=mybir.AluOpType.mult,
                )

            # ---- T-part (features 0..NI_T-1): column-oriented, den on PE
            # at base partition 0 (matmul output base must be 0/32/64,
            # and it keeps the divide partition-aligned with xt_sb) ----
            den_ps = ps_den.tile([NI_T, 2, HALF], f32)
            for h in range(2):
                eth = et_sb[:, h * HALF:(h + 1) * HALF]
                for w in range(NCH // 2):
                    sT = ps_T.tile([CW, 2, HALF], f32)
                    for j in range(2):
                        c = 2 * w + j
                        nc.tensor.matmul(
                            sT[:, j, :], w2T[:, c, :], eth,
                            start=True, stop=True,
                        )
                    expT = exptp.tile([CW, 2, HALF], bf16, name="expT")
                    sf = sT.rearrange("p a b -> p (a b)")
                    ef = expT.rearrange("p a b -> p (a b)")
                    WA = TDRAIN_ACT
                    nc.scalar.activation(
                        ef[:, 0:WA], sf[:, 0:WA],
                        mybir.ActivationFunctionType.Exp,
                        scale=1.0 / SCHRAU_A, bias=exp_bias[0:CW, :],
                    )
                    nc.vector.tensor_copy(
                        out=ef[:, WA:2 * HALF].bitcast(i16),
                        in_=sf[:, WA:2 * HALF],
                    )
                    for j in range(2):
                        c = 2 * w + j
                        nc.tensor.matmul(
                            den_ps[:, h, :], g3[:, c, :], expT[:, j, :],
                            start=(c == 0), stop=(c == NCH - 1),
                        )
            recT = outtp.tile([NI_T, MACRO], f32, name="recT")
            nc.vector.reciprocal(
                out=recT, in_=den_ps.rearrange("p a b -> p (a b)"))
            outT = outtp.tile([NI_T, MACRO], f32, name="outT")
            nc.gpsimd.tensor_tensor(
                out=outT, in0=xt_sb[0:NI_T, :].bitcast(f32), in1=recT,
                op=mybir.AluOpType.mult,
            )
            txb = ps_txb.tile([128, NBLK * NI_T], f32, name="txb")
            for b in range(NBLK):
                nc.tensor.transpose(
                    txb[:, b * NI_T:(b + 1) * NI_T],
                    outT[:, b * 128:(b + 1) * 128], ident[0:NI_T, 0:NI_T],
                )
            nc.scalar.copy(
                out=out_sb[:, :, 0:NI_T],
                in_=txb.rearrange("p (t f) -> p t f", f=NI_T),
            )

            if off == DM - 1:
                m0 = iters[git * DM]
                og = ogs.pop(git)
                nc.sync.dma_start(
                    out=y_r[m0 * NBLK:m0 * NBLK + DM * NBLK]
                    .rearrange("m p f -> p m f"),
                    in_=og,
                )

    nc.compile()
    return nc


def _prep_weights(E_W, E_b, A_W, A_b):
    E_W = np.asarray(E_W, dtype=np.float32)
    E_b = np.asarray(E_b, dtype=np.float32)
    A_W = np.asarray(A_W, dtype=np.float32)
    A_b = np.asarray(A_b, dtype=np.float32)
    w1 = np.concatenate([E_W, np.zeros((INPUT, 1), np.float32)], axis=1)
    b1 = np.concatenate([E_b, np.float32([CONST_ROW_BIAS])]).reshape(-1, 1)
    dW = A_W - A_W[:, :, 1:2]                        # [66, 50, 20]
    db = A_b - A_b[:, 1:2]                           # [66, 20]
    # (i-major, a-minor) columns; S-part = features NI_T..65 first, then
    # T-part = features 0..NI_T-1, both scaled so mm2 emits s' = A*s + B.
    dw2 = (dW.transpose(1, 0, 2) * np.float32(SCHRAU_A)).astype(np.float32)
    db2 = (db * np.float32(SCHRAU_A) + np.float32(SCHRAU_B)).astype(np.float32)
    w2i = np.concatenate([dw2, db2[None]], axis=0)   # [51, 66, 20]
    w2 = np.concatenate(
        [w2i[:, NI_T:].reshape(E_NODE + 1, SCOLS),
         w2i[:, :NI_T].reshape(E_NODE + 1, TCOLS)], axis=1)  # [51, 1320]
    # T-part group-membership matrix: G[p, c, i] = 1 iff T-col c*CW+p
    # belongs to T-feature i
    import ml_dtypes
    G = np.zeros((CW, NCH, NI_T), np.float32)
    cols = np.arange(NCH * CW)
    G[cols % CW, cols // CW, cols // A_NODE] = 1.0
    G = G.reshape(CW, -1).astype(ml_dtypes.bfloat16)
    return (np.ascontiguousarray(w1), np.ascontiguousarray(b1),
            np.ascontiguousarray(w2), np.ascontiguousarray(G))


def _run(x, E_W, E_b, A_W, A_b, trace=False):
    from concourse.bass_utils import run_bass_kernel_spmd

    x = np.ascontiguousarray(np.asarray(x, dtype=np.float32))
    n_rows_local = x.shape[0] // N_CORES
    key = ("nc", n_rows_local)
    if key not in _CACHE:
        _CACHE[key] = _build_bass(n_rows_local)
    nc = _CACHE[key]

    w1, b1, w2, G = _prep_weights(E_W, E_b, A_W, A_b)
    in_maps = [
        {"x": x[i * n_rows_local:(i + 1) * n_rows_local],
         "W1": w1, "b1": b1, "W2": w2, "G": G}
        for i in range(N_CORES)
    ]
    res = run_bass_kernel_spmd(nc, in_maps, list(range(N_CORES)), trace=trace)
    out = np.concatenate([res.results[i]["y"] for i in range(N_CORES)], axis=0)
    return out, res


def kernel(x, E_W, E_b, A_W, A_b):
    out, _ = _run(x, E_W, E_b, A_W, A_b, trace=False)
    return out


# revision 32
# speedup vs baseline: 2.8376x; 2.8376x over previous
"""Trainium2 Bass kernel for nn_AttentionModule.

Computation (per batch row b, input feature i):
    E      = tanh(x @ E_W + E_b)                      # [B, 50]
    s      = einsum('be,iea->bia', E, A_W) + A_b      # [B, 66, 20]
    A      = softmax(s, -1)[..., 1]                   # [B, 66]
    out    = x * A

Key rewrite: softmax(s)[1] = 1 / sum_a exp(s_a - s_1).  We pre-difference the
attention weights on the host (dW = A_W - A_W[:,:,1:2], db likewise), fold the
bias into the matmul via a constant-1 row of E (tanh(0*x + 30) == 1.0), and get

    den[b,i] = sum_a exp(E~ @ W2[:, a*66+i])   (a=1 slab is exactly 0 -> exp=1)
    out[b,i] = x[b,i] / den[b,i]

W2 columns are (a-major, i-minor) so that folding/reducing over a works on
contiguous 66-col slabs.

Engine split (the den pipeline is elementwise-bound; every engine gets a
share -- all ops ~1 elem/cyc/lane so balance is everything):
  - PE   : x transpose (f32r), mm1 (f32r), mm2 (f32r, 3 PSUM banks)
  - ACT  : tanh; exact exp of PSUM banks 0-1 (cols 0..879, one strided op);
           xT PSUM->SBUF copy
  - DVE  : Schraudolph exp of part of bank 2 (fused s*A+B -> int32, whose
           bit pattern IS ~exp(s) fp32, ~1.6% rms err, fine vs 2e-2 gate);
           grouped reduce over the first 20-FOLD_A a-slabs; reciprocal
  - Pool : Schraudolph exp of the rest of bank 2; contiguous fold of the
           last FOLD_A a-slabs onto slabs 0..FOLD_A-1; final x*rec multiply

Schraudolph safety: needs s >= -88 so s*A+B0 stays >= 0 (else the int32 bit
pattern goes negative = garbage float).  Real data has s in [-57, 55].
"""

import numpy as np

B_TOTAL, INPUT, E_NODE, A_NODE = 262144, 66, 50, 20
N_CORES = 8
B_LOCAL = B_TOTAL // N_CORES          # 32768
NBLK = 4                              # 128-row blocks per macro tile
MACRO = 128 * NBLK                    # 512
NMACRO = B_LOCAL // MACRO             # 64
NIA = INPUT * A_NODE                  # 1320
CHUNK = NIA // 3                      # 440 cols per PSUM bank
CONST_ROW_BIAS = 30.0                 # tanh(30) == 1.0 in fp32

SCHRAU_A = float(2**23 / np.log(2.0))  # 12102203.16
SCHRAU_B = 1064986822.0

TAIL_GROUP = 4           # blocks per reduce/recip/mul op group
DMA_MACROS = 2           # macros per x-load/y-store DMA
ACT_W = 440              # exact-exp cols per PSUM bank on ACT (of CHUNK=440);
                         # DVE Schraudolph-converts the other 440-ACT_W
XCOPY_ON = "act"         # engine for the xT PSUM->SBUF copy
MM_F32R = True           # f32r mm1 (PE 4x on it)
TX_F32R = False          # f32r transposes (walrus verifier rejects)

_CACHE = {}


def _build_bass(n_rows, repeat=1):
    import concourse.bass as bass
    import concourse.bacc as bacc
    import concourse.tile as tile
    from concourse import mybir
    from concourse.masks import make_identity
    from contextlib import ExitStack

    f32 = mybir.dt.float32
    f32r = mybir.dt.float32r
    i32 = mybir.dt.int32
    nmacro = n_rows // MACRO

    nc = bacc.Bacc("TRN2", target_bir_lowering=False, debug=False,
                   num_devices=N_CORES)

    x_d = nc.dram_tensor("x", [n_rows, INPUT], f32, kind="ExternalInput").ap()
    w1_d = nc.dram_tensor("W1", [INPUT, E_NODE + 1], f32, kind="ExternalInput").ap()
    b1_d = nc.dram_tensor("b1", [E_NODE + 1, 1], f32, kind="ExternalInput").ap()
    w2_d = nc.dram_tensor("W2", [E_NODE + 1, NIA], f32r, kind="ExternalInput").ap()
    y_d = nc.dram_tensor("y", [n_rows, INPUT], f32, kind="ExternalOutput").ap()

    x_r = x_d.rearrange("(m p) f -> m p f", p=128)
    y_r = y_d.rearrange("(m p) f -> m p f", p=128)

    with tile.TileContext(nc) as tc, ExitStack() as ctx:
        const = ctx.enter_context(tc.tile_pool(name="const", bufs=1))
        xpool = ctx.enter_context(tc.tile_pool(name="xp", bufs=3))
        xtp = ctx.enter_context(tc.tile_pool(name="xtp", bufs=2))
        etp = ctx.enter_context(tc.tile_pool(name="etp", bufs=2))
        expp = ctx.enter_context(tc.tile_pool(name="expp", bufs=6))
        denp = ctx.enter_context(tc.tile_pool(name="denp", bufs=8))
        outp = ctx.enter_context(tc.tile_pool(name="outp", bufs=3))
        ps_xt = ctx.enter_context(tc.tile_pool(name="ps_xt", bufs=1, space="PSUM"))
        ps_et = ctx.enter_context(tc.tile_pool(name="ps_et", bufs=1, space="PSUM"))
        ps_s = ctx.enter_context(tc.tile_pool(name="ps_s", bufs=2, space="PSUM"))

        w1_sb = const.tile([INPUT, E_NODE + 1], f32r if MM_F32R else f32)
        nc.sync.dma_start(out=w1_sb, in_=w1_d.bitcast(w1_sb.dtype))
        b1_sb = const.tile([E_NODE + 1, 1], f32)
        nc.sync.dma_start(out=b1_sb, in_=b1_d)
        w2_sb = const.tile([E_NODE + 1, NIA], f32r)
        nc.sync.dma_start(out=w2_sb, in_=w2_d)
        ident = const.tile([128, 128], f32)
        make_identity(nc, ident)
        ident_mm = ident.bitcast(f32r) if MM_F32R else ident
        exp_bias = const.tile([128, 1], f32)
        nc.vector.memset(exp_bias, -SCHRAU_B / SCHRAU_A)

        DM = DMA_MACROS
        assert nmacro % DM == 0
        iters = [m for _ in range(repeat) for m in range(nmacro)]
        xgs = {}

        def emit_load(git):
            """One batched x DMA covering DM consecutive macros."""
            m0 = iters[git * DM]
            xg = xpool.tile([128, DM * NBLK, INPUT], f32)
            nc.sync.dma_start(
                out=xg,
                in_=x_r[m0 * NBLK:m0 * NBLK + DM * NBLK]
                .rearrange("m p f -> p m f"),
            )
            return xg

        def emit_head(it):
            """PE transpose -> copy -> mm1 -> tanh for iteration it."""
            git, off = it // DM, it % DM
            if git not in xgs:
                xgs[git] = emit_load(git)
            x_sb = xgs[git][:, off * NBLK:(off + 1) * NBLK, :]
            xt_ps = ps_xt.tile([INPUT, MACRO], f32)
            for b in range(NBLK):
                src = x_sb[:, b, :]
                dst = xt_ps[:, b * 128:(b + 1) * 128]
                if TX_F32R:
                    src, dst = src.bitcast(f32r), dst.bitcast(f32r)
                nc.tensor.transpose(dst, src,
                                    ident_mm if TX_F32R else ident)
            xt_sb = xtp.tile([INPUT, MACRO], f32r if MM_F32R else f32)
            if XCOPY_ON == "dma":
                nc.sync.dma_start(out=xt_sb, in_=xt_ps.bitcast(xt_sb.dtype))
            elif XCOPY_ON == "act":
                nc.scalar.copy(out=xt_sb, in_=xt_ps)
            else:
                nc.vector.tensor_copy(out=xt_sb, in_=xt_ps)
            et_ps = ps_et.tile([E_NODE + 1, MACRO], f32)
            nc.tensor.matmul(et_ps, w1_sb, xt_sb, start=True, stop=True)
            et_sb = etp.tile([E_NODE + 1, MACRO], f32r)
            nc.scalar.activation(
                et_sb, et_ps, mybir.ActivationFunctionType.Tanh,
                bias=b1_sb, scale=1.0,
            )
            return x_sb, et_sb

        ogs = {}
        heads = {0: emit_head(0)}
        for it in range(len(iters)):
            m = iters[it]
            git, off = it // DM, it % DM
            if it + 1 < len(iters):
                heads[it + 1] = emit_head(it + 1)
            x_sb, et_sb = heads.pop(it)

            if git not in ogs:
                ogs[git] = outp.tile([128, DM * NBLK, INPUT], f32, name="og")
            out_sb = ogs[git][:, off * NBLK:(off + 1) * NBLK, :]
            TG = TAIL_GROUP
            exp_g = None
            for b in range(NBLK):
                bg = b % TG
                # mm2: S [128, NIA] in 3 PSUM banks (cols 0/512/1024)
                s_ps = ps_s.tile([128, 3 * 512], f32)
                lhs = et_sb[:, b * 128:(b + 1) * 128]
                for c in range(3):
                    nc.tensor.matmul(
                        s_ps[:, c * 512:c * 512 + CHUNK], lhs,
                        w2_sb[:, c * CHUNK:(c + 1) * CHUNK],
                        start=True, stop=True,
                    )

                if bg == 0:
                    exp_g = expp.tile([128, TG * NIA], f32, name="exp_g")
                exp_sb = exp_g[:, bg * NIA:(bg + 1) * NIA]

                # mm2 emitted s' = A*s + B (A,B folded into w2 on the host).
                # First ACT_W cols of each bank: exact exp on ACT (scale/
                # bias undo the affine).  Tail of each bank: Schraudolph
                # exp = bits(int32(s')) via one dtype-converting DVE copy.
                # (GPSIMD can't read PSUM, so Pool gets no drain share.)
                s3 = s_ps.rearrange("p (c w) -> p c w", w=512)
                e3 = exp_sb.rearrange("p (c w) -> p c w", w=CHUNK)
                W = ACT_W
                if W < CHUNK:
                    nc.scalar.activation(
                        e3[:, :, 0:W], s3[:, :, 0:W],
                        mybir.ActivationFunctionType.Exp,
                        scale=1.0 / SCHRAU_A, bias=exp_bias,
                    )
                else:
                    nc.scalar.activation(
                        e3[:, :, 0:W], s3[:, :, 0:W],
                        mybir.ActivationFunctionType.Exp,
                    )
                if W < CHUNK:
                    nc.vector.tensor_copy(
                        out=e3[:, :, W:CHUNK].bitcast(i32),
                        in_=s3[:, :, W:CHUNK],
                    )

                if bg != TG - 1:
                    continue

                # grouped den over a: one contiguous-innermost DVE reduce
                # ((i,a) column order -> 20-elem stride-1 groups)
                b0 = b - bg
                g = exp_g.rearrange("p (g a) -> p g a", a=A_NODE)
                den = denp.tile([128, TG * INPUT], f32, name="den")
                rec = denp.tile([128, TG * INPUT], f32, name="rec")
                nc.vector.tensor_reduce(
                    out=den, in_=g,
                    axis=mybir.AxisListType.X, op=mybir.AluOpType.add,
                )
                xs = x_sb[:, b0:b0 + TG, :]
                ys = out_sb[:, b0:b0 + TG, :]
                nc.vector.reciprocal(out=rec, in_=den)
                rec3 = rec.rearrange("p (t f) -> p t f", f=INPUT)
                nc.gpsimd.tensor_tensor(
                    out=ys, in0=xs, in1=rec3, op=mybir.AluOpType.mult,
                )

            if off == DM - 1:
                m0 = iters[git * DM]
                og = ogs.pop(git)
                nc.sync.dma_start(
                    out=y_r[m0 * NBLK:m0 * NBLK + DM * NBLK]
                    .rearrange("m p f -> p m f"),
                    in_=og,
                )

    nc.compile()
    return nc


def _prep_weights(E_W, E_b, A_W, A_b):
    E_W = np.asarray(E_W, dtype=np.float32)
    E_b = np.asarray(E_b, dtype=np.float32)
    A_W = np.asarray(A_W, dtype=np.float32)
    A_b = np.asarray(A_b, dtype=np.float32)
    w1 = np.concatenate([E_W, np.zeros((INPUT, 1), np.float32)], axis=1)
    b1 = np.concatenate([E_b, np.float32([CONST_ROW_BIAS])]).reshape(-1, 1)
    dW = A_W - A_W[:, :, 1:2]                        # [66, 50, 20]
    db = A_b - A_b[:, 1:2]                           # [66, 20]
    # (i-major, a-minor) column order: col = i*20 + a, so den groups are
    # contiguous 20-col runs.  When Schraudolph columns are in play, scale
    # by A and fold B into the bias row so mm2 emits s' = A*s + B directly
    # (f32r truncation of the ~1e9 bias then shifts ALL outputs by a small
    # common factor, which stays well inside the error budget).
    dw2 = dW.transpose(1, 0, 2).reshape(E_NODE, NIA)
    db2 = db.reshape(1, NIA)
    if ACT_W < CHUNK:
        w2 = np.concatenate(
            [dw2 * np.float32(SCHRAU_A),
             db2 * np.float32(SCHRAU_A) + np.float32(SCHRAU_B)], axis=0)
    else:
        w2 = np.concatenate([dw2, db2], axis=0)
    w2 = w2.astype(np.float32)                       # [51, 1320]
    return np.ascontiguousarray(w1), np.ascontiguousarray(b1), \
        np.ascontiguousarray(w2)


def _run(x, E_W, E_b, A_W, A_b, trace=False):
    from concourse.bass_utils import run_bass_kernel_spmd

    x = np.ascontiguousarray(np.asarray(x, dtype=np.float32))
    n_rows_local = x.shape[0] // N_CORES
    key = ("nc", n_rows_local)
    if key not in _CACHE:
        _CACHE[key] = _build_bass(n_rows_local)
    nc = _CACHE[key]

    w1, b1, w2 = _prep_weights(E_W, E_b, A_W, A_b)
    in_maps = [
        {"x": x[i * n_rows_local:(i + 1) * n_rows_local],
         "W1": w1, "b1": b1, "W2": w2}
        for i in range(N_CORES)
    ]
    res = run_bass_kernel_spmd(nc, in_maps, list(range(N_CORES)), trace=trace)
    out = np.concatenate([res.results[i]["y"] for i in range(N_CORES)], axis=0)
    return out, res


def kernel(x, E_W, E_b, A_W, A_b):
    out, _ = _run(x, E_W, E_b, A_W, A_b, trace=False)
    return out
